# revision 23
# baseline (speedup 1.0000x reference)
"""Trainium2 Bass kernel for causal multi-head attention.

Reference computation (B=2, T=2048, D=1024, H=16 heads, head_dim=64):
    q, k, v = x @ Wq, x @ Wk, x @ Wv         (per-head split)
    out = softmax(causal(q k^T / 8)) v  @ Wo

Sharding: 8 cores = 2 batches x 4 head-groups (4 heads each).  Each core
computes, for its batch b and its 4 heads:
    qT, kT [256, 2048] and v [2048, 256]  from the host-pre-transposed xT,
    transposed scores sT[tk, tq] = kT.T @ qT  (so softmax sums land on the
    matmul contraction axis and no on-chip transposes are ever needed),
    expS = exp(sT/8) * causal_mask,
    ctxT' [65, tq] = v'.T @ expS   with v' = [v | ones] so row 64 is the
    softmax denominator,
    ctxT_norm = ctxT * (1/rowsum)  (rank-1 PE broadcast of the reciprocal),
    partial_out [2048, 1024] = ctxT.T @ Wo[g*256:(g+1)*256, :].
Host sums the 4 partials per batch.

All matmuls run as float32r (TF32-like, full PE rate at N>=256).  Tiles that
feed the PE are allocated as float32r (walrus requires producer dtype to
match); PSUM accumulation stays fp32.

Scheduling: the attention i-loop rotates over all 4 heads (sT x4 then ctx x4)
so the PE never waits on a single exp, and the next chunk's QKV projection
matmuls are interleaved into the attention stream as fill work.
"""

import sys

if "/opt/trn_rl_repo" not in sys.path:
    sys.path.insert(0, "/opt/trn_rl_repo")

import numpy as np

B, T, D, H = 2, 2048, 1024, 16
HD = 64                   # head dim
NCORES = 8
GROUPS = 4                # head groups (cores per batch)
HPC = H // GROUPS         # heads per core = 4
DHC = HPC * HD            # per-core head columns = 256
NKB = D // 128            # 8 contraction blocks for the projections
NTB = T // 128            # 16 t-blocks
NCH = T // 512            # 4 tq chunks of 512

_CACHE = {}


def _build():
    import concourse.bacc as bacc
    import concourse.tile as tile
    from concourse import mybir

    fp32 = mybir.dt.float32
    fp32r = mybir.dt.float32r
    Exp = mybir.ActivationFunctionType.Exp

    nc = bacc.Bacc("TRN2", target_bir_lowering=False, debug=False,
                   num_devices=NCORES)

    xt_d = nc.dram_tensor("xt", [D, T], fp32, kind="ExternalInput")
    wq_d = nc.dram_tensor("wq", [D, DHC], fp32, kind="ExternalInput")
    wk_d = nc.dram_tensor("wk", [D, DHC], fp32, kind="ExternalInput")
    wv_d = nc.dram_tensor("wv", [D, DHC], fp32, kind="ExternalInput")
    wo_d = nc.dram_tensor("wo", [DHC, D], fp32, kind="ExternalInput")
    cm_d = nc.dram_tensor("cmask", [128, 1024], fp32, kind="ExternalInput")
    out_d = nc.dram_tensor("out", [T, D], fp32, kind="ExternalOutput")

    with tile.TileContext(nc) as tc:
        with (
            tc.tile_pool(name="consts", bufs=1) as consts,
            tc.tile_pool(name="xtp", bufs=2) as xtp,
            tc.tile_pool(name="big", bufs=1) as big,
            tc.tile_pool(name="es_pool", bufs=8) as es_pool,
            tc.tile_pool(name="small", bufs=3) as small,
            tc.tile_pool(name="outp", bufs=2) as outp,
            tc.tile_pool(name="psum", bufs=1, space="PSUM") as psum,
        ):
            wq_sb = consts.tile([128, NKB, DHC], fp32r)
            wk_sb = consts.tile([128, NKB, DHC], fp32r)
            wv_sb = consts.tile([128, NKB, DHC], fp32r)
            wo_sb = consts.tile([128, 2, D], fp32r)
            cm_sb = consts.tile([128, 1024], fp32r)
            nc.sync.dma_start(out=wq_sb, in_=wq_d[:].rearrange("(k p) n -> p k n", p=128).bitcast(fp32r))

            qt_sb = big.tile([128, 2, T], fp32r)
            kt_sb = big.tile([128, 2, T], fp32r)
            ct_sb = big.tile([128, 2, T], fp32r)
            vs_sb = big.tile([128, NTB, HPC, HD + 1], fp32r)

            xt_r = xt_d[:].rearrange("(k p) t -> p k t", p=128).bitcast(fp32r)
            xt_c = [None] * NCH
            # pcS[nj][h]: ctxT' drained to SBUF at end of chunk nj's attention
            pcS = [[None] * HPC for _ in range(NCH)]
            rsAll = [None] * NCH

            def load_xt(nj):
                xt_c[nj] = xtp.tile([128, NKB, 512], fp32r, tag="xt",
                                    name=f"xt{nj}")
                half = NKB // 2
                nc.sync.dma_start(out=xt_c[nj][:, 0:half, :],
                                  in_=xt_r[:, 0:half, nj * 512:(nj + 1) * 512])
                nc.sync.dma_start(out=xt_c[nj][:, half:, :],
                                  in_=xt_r[:, half:, nj * 512:(nj + 1) * 512])

            def qkv_halves(nj):
                """16 closures, each half a psum accumulation group (4 MMs)."""
                cs = slice(nj * 512, (nj + 1) * 512)
                quanta = []

                def make_qk(wsb, dst, mb):
                    pq = [None]

                    def go_a():
                        pq[0] = psum.tile([128, 512], fp32, tag="aux", bufs=1,
                                          name=f"pq{nj}{mb}")
                        for kb in range(4):
                            nc.tensor.matmul(
                                pq[0],
                                wsb[:, kb, mb * 128:(mb + 1) * 128],
                                xt_c[nj][:, kb, :],
                                start=(kb == 0), stop=False,
                            )

                    def go_b():
                        for kb in range(4, NKB):
                            nc.tensor.matmul(
                                pq[0],
                                wsb[:, kb, mb * 128:(mb + 1) * 128],
                                xt_c[nj][:, kb, :],
                                start=False, stop=(kb == NKB - 1),
                            )
                        nc.vector.tensor_copy(dst[:, mb, cs], pq[0])
                    return go_a, go_b

                def make_v(tb):
                    pv = [None]

                    def go_a():
                        pv[0] = psum.tile([128, 512], fp32, tag="aux", bufs=1,
                                          name=f"pv{tb}")
                        for kb in range(4):
                            nc.tensor.matmul(
                                pv[0][:, 0:DHC],
                                xt_c[nj][:, kb, (tb - 4 * nj) * 128:(tb - 4 * nj + 1) * 128],
                                wv_sb[:, kb, :],
                                start=(kb == 0), stop=False,
                            )

                    def go_b():
                        for kb in range(4, NKB):
                            nc.tensor.matmul(
                                pv[0][:, 0:DHC],
                                xt_c[nj][:, kb, (tb - 4 * nj) * 128:(tb - 4 * nj + 1) * 128],
                                wv_sb[:, kb, :],
                                start=False, stop=(kb == NKB - 1),
                            )
                        nc.vector.tensor_copy(
                            vs_sb[:, tb, :, 0:HD],
                            pv[0][:, 0:DHC].rearrange("p (h d) -> p h d", h=HPC),
                        )
                    return go_a, go_b

                for mb in range(2):
                    quanta.extend(make_qk(wq_sb, qt_sb, mb))
                for mb in range(2):
                    quanta.extend(make_qk(wk_sb, kt_sb, mb))
                for tb in range(4 * nj, 4 * nj + 4):
                    quanta.extend(make_v(tb))
                return quanta

            rcAll = [None] * NCH

            def prenorm_fill(nj):
                """two batched reciprocals cover all 4 heads' rowsums
                (heads live at 32-aligned partitions 0/32 of two tiles)."""
                def go():
                    rcAll[nj] = []
                    for half in range(2):
                        tmp32 = small.tile([33, 512], fp32, tag="tmp32",
                                           bufs=2, name=f"tmp{nj}{half}")
                        nc.vector.reciprocal(out=tmp32, in_=rsAll[nj][half])
                        for sub in range(2):
                            rch = small.tile([1, 512], fp32r, tag="rc", bufs=8,
                                             name=f"rc{nj}{half}{sub}")
                            with nc.allow_low_precision(reason="fp32r recip for PE bcast"):
                                nc.vector.tensor_copy(
                                    rch, tmp32[32 * sub:32 * sub + 1, :])
                            rcAll[nj].append(rch)
                return go

            def norm_fill(nj, h):
                """normalize head h of chunk nj from the SBUF-drained ctxT'."""
                def go():
                    mbh, ro = h >> 1, (h & 1) * 64
                    src = pcS[nj][h]
                    pb = psum.tile([64, 512], fp32, tag="aux", bufs=1,
                                   name=f"pb{nj}{h}")
                    nc.tensor.matmul(pb, cm_sb[0:1, 512:576], rcAll[nj][h],
                                     start=True, stop=True)
                    nc.vector.tensor_mul(
                        ct_sb[ro:ro + 64, mbh, nj * 512:(nj + 1) * 512],
                        src[0:64, :], pb)
                return go

            def outproj_fill(nj, tb):
                def go():
                    ot = outp.tile([128, D], fp32, tag="ot", name=f"ot{tb}")
                    for nk in range(2):
                        po = psum.tile([128, 512], fp32, tag="aux", bufs=1,
                                       name=f"po{tb}{nk}")
                        for mb in range(2):
                            nc.tensor.matmul(
                                po,
                                ct_sb[:, mb, tb * 128:(tb + 1) * 128],
                                wo_sb[:, mb, nk * 512:(nk + 1) * 512],
                                start=(mb == 0), stop=(mb == 1),
                            )
                        nc.vector.tensor_copy(ot[:, nk * 512:(nk + 1) * 512], po)
                    nc.sync.dma_start(out=out_d[tb * 128:(tb + 1) * 128, :],
                                      in_=ot)
                return go

            def norm_fills(nj):
                return [prenorm_fill(nj)] + [norm_fill(nj, h)
                                             for h in range(HPC)]

            def outproj_fills(nj):
                return [outproj_fill(nj, tb)
                        for tb in range(4 * nj, 4 * nj + 4)]

            # prologue: DMAs ordered so QKV(0) can start as early as
            # possible (wq then xt0 land first), then first chunk's QKV
            load_xt(0)
            nc.sync.dma_start(out=wk_sb, in_=wk_d[:].rearrange("(k p) n -> p k n", p=128).bitcast(fp32r))
            nc.sync.dma_start(out=wv_sb, in_=wv_d[:].rearrange("(k p) n -> p k n", p=128).bitcast(fp32r))
            load_xt(1)
            nc.sync.dma_start(out=cm_sb, in_=cm_d[:].bitcast(fp32r))
            nc.sync.dma_start(out=wo_sb, in_=wo_d[:].rearrange("(k p) n -> p k n", p=128).bitcast(fp32r))
            # ones column of v' (cmask cols 512.. are all 1.0, dtype fp32r)
            nc.vector.tensor_copy(
                vs_sb[:, :, :, 64],
                cm_sb[:, 512:512 + NTB * HPC].rearrange("p (a b) -> p a b", a=NTB),
            )
            for q in qkv_halves(0):
                q()

            for nj in range(NCH):
                nb = 4 * nj + 4     # causal: tk-blocks 0 .. nb-1
                if nj + 2 < NCH:
                    load_xt(nj + 2)
                # deferred fill work for this chunk's attention span:
                #   - next chunk's QKV projections
                #   - previous chunk's normalization
                #   - output projections delayed by TWO chunks (they have no
                #     downstream consumers, so park them where attention has
                #     the least other fill -- the late, widest chunks)
                fill = list(qkv_halves(nj + 1)) if nj + 1 < NCH else []
                if nj >= 1:
                    # norm after ALL qkv fills: its 3.3us DVE reciprocal must
                    # not sit in the DVE queue ahead of load-bearing copies
                    fill = fill + norm_fills(nj - 1)
                if nj == NCH - 2:
                    fill += outproj_fills(0)
                if nj == NCH - 1:
                    fill += outproj_fills(NCH - 3) + outproj_fills(NCH - 2)
                fi = 0

                pcs = []
                for h in range(HPC):
                    pc = psum.tile([65, 512], fp32, tag="acc", bufs=4,
                                   name=f"pc{nj}{h}")
                    pcs.append(pc)

                pace = len(fill) / nb
                for i in range(nb):
                    m = i - 4 * nj
                    # causal window: diagonal blocks only need cols >= wm
                    # (m==3 keeps N>=256 to stay at full fp32r rate)
                    wm = 0 if m < 0 else (128 * m if m < 3 else 256)
                    ess = [None] * HPC
                    for hp in range(2):
                        pss = []
                        for h in (2 * hp, 2 * hp + 1):
                            mbh, ro = h >> 1, (h & 1) * 64
                            ps = psum.tile([128, 512], fp32, tag="ps", bufs=3,
                                           name=f"ps{nj}{h}{i}")
                            nc.tensor.matmul(
                                ps[:, wm:512],
                                kt_sb[ro:ro + 64, mbh, i * 128:(i + 1) * 128],
                                qt_sb[ro:ro + 64, mbh, nj * 512 + wm:(nj + 1) * 512],
                                start=True, stop=True,
                            )
                            pss.append(ps)
                        for h in (2 * hp, 2 * hp + 1):
                            ps = pss[h & 1]
                            es = es_pool.tile([128, 512], fp32r, tag="es",
                                              name=f"es{nj}{h}{i}")
                            nc.scalar.activation(out=es[:, wm:512],
                                                 in_=ps[:, wm:512],
                                                 func=Exp, scale=0.125)
                            if m >= 0:
                                # only the diagonal 128 cols (plus, for m==3,
                                # the below-window cols) need masking
                                a = wm if m == 3 else 128 * m
                                nc.vector.tensor_mul(
                                    es[:, a:128 * m + 128],
                                    es[:, a:128 * m + 128],
                                    cm_sb[:, (3 - m) * 128 + a:512],
                                )
                            ess[h] = es
                    for h in range(HPC):
                        nc.tensor.matmul(
                            pcs[h][:, wm:512],
                            vs_sb[:, i, h, :],
                            ess[h][:, wm:512],
                            start=(i == 0), stop=(i == nb - 1),
                        )
                    # fill the ACT-bound pipeline with deferred + QKV work
                    while fi < min(len(fill), int(pace * (i + 1) + 0.999)):
                        fill[fi]()
                        fi += 1
                while fi < len(fill):
                    fill[fi]()
                    fi += 1

                # drain ctxT' + rowsums to SBUF so the PSUM acc banks free
                # up and normalization can run as fill work in the next chunk
                rsAll[nj] = []
                for half in range(2):
                    rsh = small.tile([33, 512], fp32, tag="rsall", bufs=4,
                                     name=f"rsAll{nj}{half}")
                    nc.gpsimd.memset(rsh, 1.0)
                    rsAll[nj].append(rsh)
                for h in range(HPC):
                    dst = small.tile([64, 512], fp32, tag="pcs", bufs=8,
                                     name=f"pcS{nj}{h}")
                    nc.scalar.copy(dst, pcs[h][0:64, :])
                    pcS[nj][h] = dst
                    nc.vector.tensor_copy(
                        rsAll[nj][h >> 1][32 * (h & 1):32 * (h & 1) + 1, :],
                        pcs[h][64:65, :])

            # last chunk's tail has no next attention to hide in
            for go in norm_fills(NCH - 1) + outproj_fills(NCH - 1):
                go()

    nc.compile()
    return nc


def _causal_mask_block():
    # [128, 1024]: cols 0..383 = 0, cols 384..511 = upper-tri (p <= c-384),
    # cols 512.. = 1.  Slice [(3-m)*128 : (3-m)*128+512] masks a diagonal
    # tk-block at position m within a 512-wide tq chunk.
    m = np.zeros((128, 1024), np.float32)
    m[:, 512:] = 1.0
    m[:, 384:512] = np.triu(np.ones((128, 128), np.float32))
    return m


def _prepare_in_maps(x_q, Wq, Wk, Wv, Wo):
    x_q = np.asarray(x_q, np.float32)
    Wq = np.asarray(Wq, np.float32)
    Wk = np.asarray(Wk, np.float32)
    Wv = np.asarray(Wv, np.float32)
    Wo = np.asarray(Wo, np.float32)

    cmask = _causal_mask_block()
    xts = [np.ascontiguousarray(x_q[b].T) for b in range(B)]
    in_maps = []
    for c in range(NCORES):
        b, g = divmod(c, GROUPS)
        sl = slice(g * DHC, (g + 1) * DHC)
        in_maps.append({
            "xt": xts[b],
            "wq": np.ascontiguousarray(Wq[:, sl]),
            "wk": np.ascontiguousarray(Wk[:, sl]),
            "wv": np.ascontiguousarray(Wv[:, sl]),
            "wo": np.ascontiguousarray(Wo[sl, :]),
            "cmask": cmask,
        })
    return in_maps


def _gather(results):
    out = np.zeros((B, T, D), np.float32)
    for c in range(NCORES):
        out[c // GROUPS] += results[c]["out"]
    return out


def get_nc():
    if "nc" not in _CACHE:
        _CACHE["nc"] = _build()
    return _CACHE["nc"]


def kernel(x_q, Wq, Wk, Wv, Wo):
    from concourse.bass_utils import run_bass_kernel_spmd

    nc = get_nc()
    in_maps = _prepare_in_maps(x_q, Wq, Wk, Wv, Wo)
    res = run_bass_kernel_spmd(nc, in_maps, list(range(NCORES)))
    return _gather(res.results)


# revision 25
# speedup vs baseline: 1.1753x; 1.1753x over previous
"""Trainium2 Bass kernel for causal multi-head attention.

Reference computation (B=2, T=2048, D=1024, H=16 heads, head_dim=64):
    q, k, v = x @ Wq, x @ Wk, x @ Wv         (per-head split)
    out = softmax(causal(q k^T / 8)) v  @ Wo

Sharding: 8 cores = 2 batches x 4 head-groups (4 heads each).  Each core
computes, for its batch b and its 4 heads:
    qT, kT [256, 2048] and v [2048, 256]  from the host-pre-transposed xT,
    transposed scores sT[tk, tq] = kT.T @ qT  (so softmax sums land on the
    matmul contraction axis and no on-chip transposes are ever needed),
    expS = exp(sT/8) * causal_mask,
    ctxT' [65, tq] = v'.T @ expS   with v' = [v | ones] so row 64 is the
    softmax denominator,
    ctxT_norm = ctxT * (1/rowsum)  (rank-1 PE broadcast of the reciprocal),
    partial_out [2048, 1024] = ctxT.T @ Wo[g*256:(g+1)*256, :].
Host sums the 4 partials per batch.

All matmuls run as float32r (TF32-like, full PE rate at N>=256).  Tiles that
feed the PE are allocated as float32r (walrus requires producer dtype to
match); PSUM accumulation stays fp32.

Scheduling: the attention i-loop rotates over all 4 heads (sT x4 then ctx x4)
so the PE never waits on a single exp, and the next chunk's QKV projection
matmuls are interleaved into the attention stream as fill work.
"""

import sys

if "/opt/trn_rl_repo" not in sys.path:
    sys.path.insert(0, "/opt/trn_rl_repo")

import numpy as np

B, T, D, H = 2, 2048, 1024, 16
HD = 64                   # head dim
NCORES = 8
GROUPS = 4                # head groups (cores per batch)
HPC = H // GROUPS         # heads per core = 4
DHC = HPC * HD            # per-core head columns = 256
NKB = D // 128            # 8 contraction blocks for the projections
NTB = T // 128            # 16 t-blocks
NCH = T // 512            # 4 tq chunks of 512

_CACHE = {}


def _build():
    import concourse.bacc as bacc
    import concourse.tile as tile
    from concourse import mybir

    fp32 = mybir.dt.float32
    fp32r = mybir.dt.float32r
    Exp = mybir.ActivationFunctionType.Exp

    nc = bacc.Bacc("TRN2", target_bir_lowering=False, debug=False,
                   num_devices=NCORES)

    xt_d = nc.dram_tensor("xt", [D, T], fp32, kind="ExternalInput")
    wq_d = nc.dram_tensor("wq", [D, DHC], fp32, kind="ExternalInput")
    wk_d = nc.dram_tensor("wk", [D, DHC], fp32, kind="ExternalInput")
    wv_d = nc.dram_tensor("wv", [D, DHC], fp32, kind="ExternalInput")
    wo_d = nc.dram_tensor("wo", [DHC, D], fp32, kind="ExternalInput")
    cm_d = nc.dram_tensor("cmask", [128, 1024], fp32, kind="ExternalInput")
    out_d = nc.dram_tensor("out", [T, D], fp32, kind="ExternalOutput")

    with tile.TileContext(nc) as tc:
        with (
            tc.tile_pool(name="consts", bufs=1) as consts,
            tc.tile_pool(name="xtp", bufs=2) as xtp,
            tc.tile_pool(name="big", bufs=1) as big,
            tc.tile_pool(name="es_pool", bufs=8) as es_pool,
            tc.tile_pool(name="small", bufs=3) as small,
            tc.tile_pool(name="outp", bufs=2) as outp,
            tc.tile_pool(name="psum", bufs=1, space="PSUM") as psum,
        ):
            wq_sb = consts.tile([128, NKB, DHC], fp32r)
            wk_sb = consts.tile([128, NKB, DHC], fp32r)
            wv_sb = consts.tile([128, NKB, DHC], fp32r)
            wo_sb = consts.tile([128, 2, D], fp32r)
            cm_sb = consts.tile([128, 1024], fp32r)
            nc.sync.dma_start(out=wq_sb, in_=wq_d[:].rearrange("(k p) n -> p k n", p=128).bitcast(fp32r))

            qt_sb = big.tile([128, 2, T], fp32r)
            kt_sb = big.tile([128, 2, T], fp32r)
            ct_sb = big.tile([128, 2, T], fp32r)
            vs_sb = big.tile([128, NTB, HPC, HD + 1], fp32r)

            xt_r = xt_d[:].rearrange("(k p) t -> p k t", p=128).bitcast(fp32r)
            xt_c = [None] * NCH
            # pcS[nj][h]: ctxT' drained to SBUF at end of chunk nj's attention
            pcS = [[None] * HPC for _ in range(NCH)]
            rsAll = [None] * NCH

            def load_xt(nj):
                xt_c[nj] = xtp.tile([128, NKB, 512], fp32r, tag="xt",
                                    name=f"xt{nj}")
                half = NKB // 2
                nc.sync.dma_start(out=xt_c[nj][:, 0:half, :],
                                  in_=xt_r[:, 0:half, nj * 512:(nj + 1) * 512])
                nc.sync.dma_start(out=xt_c[nj][:, half:, :],
                                  in_=xt_r[:, half:, nj * 512:(nj + 1) * 512])

            def qkv_halves(nj):
                """16 closures, each half a psum accumulation group (4 MMs)."""
                cs = slice(nj * 512, (nj + 1) * 512)
                quanta = []

                def make_qk(wsb, dst, mb):
                    pq = [None]

                    def go_a():
                        pq[0] = psum.tile([128, 512], fp32, tag="mm", bufs=4,
                                          name=f"pq{nj}{mb}")
                        for kb in range(4):
                            nc.tensor.matmul(
                                pq[0],
                                wsb[:, kb, mb * 128:(mb + 1) * 128],
                                xt_c[nj][:, kb, :],
                                start=(kb == 0), stop=False,
                            )

                    def go_b():
                        for kb in range(4, NKB):
                            nc.tensor.matmul(
                                pq[0],
                                wsb[:, kb, mb * 128:(mb + 1) * 128],
                                xt_c[nj][:, kb, :],
                                start=False, stop=(kb == NKB - 1),
                            )
                        nc.vector.tensor_copy(dst[:, mb, cs], pq[0])
                    return go_a, go_b

                def make_v(tb):
                    pv = [None]

                    def go_a():
                        pv[0] = psum.tile([128, 512], fp32, tag="mm", bufs=4,
                                          name=f"pv{tb}")
                        for kb in range(4):
                            nc.tensor.matmul(
                                pv[0][:, 0:DHC],
                                xt_c[nj][:, kb, (tb - 4 * nj) * 128:(tb - 4 * nj + 1) * 128],
                                wv_sb[:, kb, :],
                                start=(kb == 0), stop=False,
                            )

                    def go_b():
                        for kb in range(4, NKB):
                            nc.tensor.matmul(
                                pv[0][:, 0:DHC],
                                xt_c[nj][:, kb, (tb - 4 * nj) * 128:(tb - 4 * nj + 1) * 128],
                                wv_sb[:, kb, :],
                                start=False, stop=(kb == NKB - 1),
                            )
                        nc.vector.tensor_copy(
                            vs_sb[:, tb, :, 0:HD],
                            pv[0][:, 0:DHC].rearrange("p (h d) -> p h d", h=HPC),
                        )
                    return go_a, go_b

                for mb in range(2):
                    quanta.extend(make_qk(wq_sb, qt_sb, mb))
                for mb in range(2):
                    quanta.extend(make_qk(wk_sb, kt_sb, mb))
                for tb in range(4 * nj, 4 * nj + 4):
                    quanta.extend(make_v(tb))
                return quanta

            rcAll = [None] * NCH

            def prenorm_fill(nj):
                """two batched reciprocals cover all 4 heads' rowsums
                (heads live at 32-aligned partitions 0/32 of two tiles)."""
                def go():
                    rcAll[nj] = []
                    for half in range(2):
                        tmp32 = small.tile([33, 512], fp32, tag="tmp32",
                                           bufs=2, name=f"tmp{nj}{half}")
                        nc.vector.reciprocal(out=tmp32, in_=rsAll[nj][half])
                        for sub in range(2):
                            rch = small.tile([1, 512], fp32r, tag="rc", bufs=8,
                                             name=f"rc{nj}{half}{sub}")
                            with nc.allow_low_precision(reason="fp32r recip for PE bcast"):
                                nc.vector.tensor_copy(
                                    rch, tmp32[32 * sub:32 * sub + 1, :])
                            rcAll[nj].append(rch)
                return go

            def norm_fill(nj, h):
                """normalize head h of chunk nj from the SBUF-drained ctxT'."""
                def go():
                    mbh, ro = h >> 1, (h & 1) * 64
                    src = pcS[nj][h]
                    pb = psum.tile([64, 512], fp32, tag="mm", bufs=4,
                                   name=f"pb{nj}{h}")
                    nc.tensor.matmul(pb, cm_sb[0:1, 512:576], rcAll[nj][h],
                                     start=True, stop=True)
                    nc.vector.tensor_mul(
                        ct_sb[ro:ro + 64, mbh, nj * 512:(nj + 1) * 512],
                        src[0:64, :], pb)
                return go

            def outproj_fill(nj, tb):
                def go():
                    ot = outp.tile([128, D], fp32, tag="ot", name=f"ot{tb}")
                    for nk in range(2):
                        po = psum.tile([128, 512], fp32, tag="mm", bufs=4,
                                       name=f"po{tb}{nk}")
                        for mb in range(2):
                            nc.tensor.matmul(
                                po,
                                ct_sb[:, mb, tb * 128:(tb + 1) * 128],
                                wo_sb[:, mb, nk * 512:(nk + 1) * 512],
                                start=(mb == 0), stop=(mb == 1),
                            )
                        nc.vector.tensor_copy(ot[:, nk * 512:(nk + 1) * 512], po)
                    nc.sync.dma_start(out=out_d[tb * 128:(tb + 1) * 128, :],
                                      in_=ot)
                return go

            def norm_fills(nj):
                return [prenorm_fill(nj)] + [norm_fill(nj, h)
                                             for h in range(HPC)]

            def outproj_fills(nj):
                return [outproj_fill(nj, tb)
                        for tb in range(4 * nj, 4 * nj + 4)]

            # prologue: DMAs ordered so QKV(0) can start as early as
            # possible (wq then xt0 land first), then first chunk's QKV
            load_xt(0)
            nc.sync.dma_start(out=wk_sb, in_=wk_d[:].rearrange("(k p) n -> p k n", p=128).bitcast(fp32r))
            nc.sync.dma_start(out=wv_sb, in_=wv_d[:].rearrange("(k p) n -> p k n", p=128).bitcast(fp32r))
            load_xt(1)
            nc.sync.dma_start(out=cm_sb, in_=cm_d[:].bitcast(fp32r))
            nc.sync.dma_start(out=wo_sb, in_=wo_d[:].rearrange("(k p) n -> p k n", p=128).bitcast(fp32r))
            # ones column of v' (cmask cols 512.. are all 1.0, dtype fp32r)
            nc.vector.tensor_copy(
                vs_sb[:, :, :, 64],
                cm_sb[:, 512:512 + NTB * HPC].rearrange("p (a b) -> p a b", a=NTB),
            )
            for q in qkv_halves(0):
                q()

            for nj in range(NCH):
                nb = 4 * nj + 4     # causal: tk-blocks 0 .. nb-1
                if nj + 2 < NCH:
                    load_xt(nj + 2)
                # deferred fill work for this chunk's attention span:
                #   - next chunk's QKV projections
                #   - previous chunk's normalization
                #   - output projections delayed by TWO chunks (they have no
                #     downstream consumers, so park them where attention has
                #     the least other fill -- the late, widest chunks)
                fill = list(qkv_halves(nj + 1)) if nj + 1 < NCH else []
                if nj >= 1:
                    # norm after ALL qkv fills: its 3.3us DVE reciprocal must
                    # not sit in the DVE queue ahead of load-bearing copies
                    fill = fill + norm_fills(nj - 1)
                if nj == NCH - 2:
                    fill += outproj_fills(0)
                if nj == NCH - 1:
                    fill += outproj_fills(NCH - 3) + outproj_fills(NCH - 2)
                fi = 0

                total_iters = 2 * nb
                it = 0
                rsAll[nj] = []
                for hp in range(2):
                    heads = (2 * hp, 2 * hp + 1)
                    pcs2 = [psum.tile([65, 512], fp32, tag="acc", bufs=2,
                                      name=f"pc{nj}{h}") for h in heads]
                    prev = None
                    for i in range(nb):
                        m = i - 4 * nj
                        # causal window: diagonal blocks only need cols >= wm
                        # (m==3 keeps N>=256 to stay at full fp32r rate)
                        wm = 0 if m < 0 else (128 * m if m < 3 else 256)
                        cur = []
                        for k, h in enumerate(heads):
                            mbh, ro = h >> 1, (h & 1) * 64
                            ps = psum.tile([128, 512], fp32, tag="mm", bufs=4,
                                           name=f"ps{nj}{h}{i}")
                            nc.tensor.matmul(
                                ps[:, wm:512],
                                kt_sb[ro:ro + 64, mbh, i * 128:(i + 1) * 128],
                                qt_sb[ro:ro + 64, mbh, nj * 512 + wm:(nj + 1) * 512],
                                start=True, stop=True,
                            )
                            es = es_pool.tile([128, 512], fp32r, tag="es",
                                              name=f"es{nj}{h}{i}")
                            nc.scalar.activation(out=es[:, wm:512],
                                                 in_=ps[:, wm:512],
                                                 func=Exp, scale=0.125)
                            if m >= 0:
                                # only the diagonal 128 cols (plus, for m==3,
                                # the below-window cols) need masking
                                a = wm if m == 3 else 128 * m
                                nc.vector.tensor_mul(
                                    es[:, a:128 * m + 128],
                                    es[:, a:128 * m + 128],
                                    cm_sb[:, (3 - m) * 128 + a:512],
                                )
                            cur.append(es)
                        if prev is not None:
                            # ctx for iteration i-1: its exps had a full
                            # iteration of PE work to complete on ACT
                            pes, pwm, pi = prev
                            for k, h in enumerate(heads):
                                nc.tensor.matmul(
                                    pcs2[k][:, pwm:512],
                                    vs_sb[:, pi, h, :],
                                    pes[k][:, pwm:512],
                                    start=(pi == 0), stop=False,
                                )
                        prev = (cur, wm, i)
                        it += 1
                        while fi < min(len(fill),
                                       int(len(fill) * it / total_iters + 0.999)):
                            fill[fi]()
                            fi += 1
                    pes, pwm, pi = prev
                    for k, h in enumerate(heads):
                        nc.tensor.matmul(
                            pcs2[k][:, pwm:512],
                            vs_sb[:, pi, h, :],
                            pes[k][:, pwm:512],
                            start=(pi == 0), stop=True,
                        )
                    # drain this pass's ctxT' + rowsums to SBUF (ACT does the
                    # big copies; rowsums go to 32-aligned rows for the
                    # batched reciprocal)
                    rsh = small.tile([33, 512], fp32, tag="rsall", bufs=4,
                                     name=f"rsAll{nj}{hp}")
                    nc.gpsimd.memset(rsh, 1.0)
                    rsAll[nj].append(rsh)
                    for k, h in enumerate(heads):
                        dst = small.tile([64, 512], fp32, tag="pcs", bufs=8,
                                         name=f"pcS{nj}{h}")
                        nc.scalar.copy(dst, pcs2[k][0:64, :])
                        pcS[nj][h] = dst
                        nc.vector.tensor_copy(rsh[32 * k:32 * k + 1, :],
                                              pcs2[k][64:65, :])
                while fi < len(fill):
                    fill[fi]()
                    fi += 1

            # last chunk's tail has no next attention to hide in
            for go in norm_fills(NCH - 1) + outproj_fills(NCH - 1):
                go()

    nc.compile()
    return nc


def _causal_mask_block():
    # [128, 1024]: cols 0..383 = 0, cols 384..511 = upper-tri (p <= c-384),
    # cols 512.. = 1.  Slice [(3-m)*128 : (3-m)*128+512] masks a diagonal
    # tk-block at position m within a 512-wide tq chunk.
    m = np.zeros((128, 1024), np.float32)
    m[:, 512:] = 1.0
    m[:, 384:512] = np.triu(np.ones((128, 128), np.float32))
    return m


def _prepare_in_maps(x_q, Wq, Wk, Wv, Wo):
    x_q = np.asarray(x_q, np.float32)
    Wq = np.asarray(Wq, np.float32)
    Wk = np.asarray(Wk, np.float32)
    Wv = np.asarray(Wv, np.float32)
    Wo = np.asarray(Wo, np.float32)

    cmask = _causal_mask_block()
    xts = [np.ascontiguousarray(x_q[b].T) for b in range(B)]
    in_maps = []
    for c in range(NCORES):
        b, g = divmod(c, GROUPS)
        sl = slice(g * DHC, (g + 1) * DHC)
        in_maps.append({
            "xt": xts[b],
            "wq": np.ascontiguousarray(Wq[:, sl]),
            "wk": np.ascontiguousarray(Wk[:, sl]),
            "wv": np.ascontiguousarray(Wv[:, sl]),
            "wo": np.ascontiguousarray(Wo[sl, :]),
            "cmask": cmask,
        })
    return in_maps


def _gather(results):
    out = np.zeros((B, T, D), np.float32)
    for c in range(NCORES):
        out[c // GROUPS] += results[c]["out"]
    return out


def get_nc():
    if "nc" not in _CACHE:
        _CACHE["nc"] = _build()
    return _CACHE["nc"]


def kernel(x_q, Wq, Wk, Wv, Wo):
    from concourse.bass_utils import run_bass_kernel_spmd

    nc = get_nc()
    in_maps = _prepare_in_maps(x_q, Wq, Wk, Wv, Wo)
    res = run_bass_kernel_spmd(nc, in_maps, list(range(NCORES)))
    return _gather(res.results)


# revision 26
# speedup vs baseline: 1.1925x; 1.0146x over previous
"""Trainium2 Bass kernel for causal multi-head attention.

Reference computation (B=2, T=2048, D=1024, H=16 heads, head_dim=64):
    q, k, v = x @ Wq, x @ Wk, x @ Wv         (per-head split)
    out = softmax(causal(q k^T / 8)) v  @ Wo

Sharding: 8 cores = 2 batches x 4 head-groups (4 heads each).  Each core
computes, for its batch b and its 4 heads:
    qT, kT [256, 2048] and v [2048, 256]  from the host-pre-transposed xT,
    transposed scores sT[tk, tq] = kT.T @ qT  (so softmax sums land on the
    matmul contraction axis and no on-chip transposes are ever needed),
    expS = exp(sT/8) * causal_mask,
    ctxT' [65, tq] = v'.T @ expS   with v' = [v | ones] so row 64 is the
    softmax denominator,
    ctxT_norm = ctxT * (1/rowsum)  (rank-1 PE broadcast of the reciprocal),
    partial_out [2048, 1024] = ctxT.T @ Wo[g*256:(g+1)*256, :].
Host sums the 4 partials per batch.

All matmuls run as float32r (TF32-like, full PE rate at N>=256).  Tiles that
feed the PE are allocated as float32r (walrus requires producer dtype to
match); PSUM accumulation stays fp32.

Scheduling: the attention i-loop rotates over all 4 heads (sT x4 then ctx x4)
so the PE never waits on a single exp, and the next chunk's QKV projection
matmuls are interleaved into the attention stream as fill work.
"""

import sys

if "/opt/trn_rl_repo" not in sys.path:
    sys.path.insert(0, "/opt/trn_rl_repo")

import numpy as np

B, T, D, H = 2, 2048, 1024, 16
HD = 64                   # head dim
NCORES = 8
GROUPS = 4                # head groups (cores per batch)
HPC = H // GROUPS         # heads per core = 4
DHC = HPC * HD            # per-core head columns = 256
NKB = D // 128            # 8 contraction blocks for the projections
NTB = T // 128            # 16 t-blocks
NCH = T // 512            # 4 tq chunks of 512

_CACHE = {}


def _build():
    import concourse.bacc as bacc
    import concourse.tile as tile
    from concourse import mybir

    fp32 = mybir.dt.float32
    fp32r = mybir.dt.float32r
    Exp = mybir.ActivationFunctionType.Exp

    nc = bacc.Bacc("TRN2", target_bir_lowering=False, debug=False,
                   num_devices=NCORES)

    xt_d = nc.dram_tensor("xt", [D, T], fp32, kind="ExternalInput")
    wq_d = nc.dram_tensor("wq", [D, DHC], fp32, kind="ExternalInput")
    wk_d = nc.dram_tensor("wk", [D, DHC], fp32, kind="ExternalInput")
    wv_d = nc.dram_tensor("wv", [D, DHC], fp32, kind="ExternalInput")
    wo_d = nc.dram_tensor("wo", [DHC, D], fp32, kind="ExternalInput")
    cm_d = nc.dram_tensor("cmask", [128, 1024], fp32, kind="ExternalInput")
    out_d = nc.dram_tensor("out", [T, D], fp32, kind="ExternalOutput")

    with tile.TileContext(nc) as tc:
        with (
            tc.tile_pool(name="consts", bufs=1) as consts,
            tc.tile_pool(name="xtp", bufs=2) as xtp,
            tc.tile_pool(name="big", bufs=1) as big,
            tc.tile_pool(name="es_pool", bufs=8) as es_pool,
            tc.tile_pool(name="small", bufs=3) as small,
            tc.tile_pool(name="outp", bufs=2) as outp,
            tc.tile_pool(name="psum", bufs=1, space="PSUM") as psum,
        ):
            wq_sb = consts.tile([128, NKB, DHC], fp32r)
            wk_sb = consts.tile([128, NKB, DHC], fp32r)
            wv_sb = consts.tile([128, NKB, DHC], fp32r)
            wo_sb = consts.tile([128, 2, D], fp32r)
            cm_sb = consts.tile([128, 1024], fp32r)
            nc.sync.dma_start(out=wq_sb, in_=wq_d[:].rearrange("(k p) n -> p k n", p=128).bitcast(fp32r))

            qt_sb = big.tile([128, 2, T], fp32r)
            kt_sb = big.tile([128, 2, T], fp32r)
            ct_sb = big.tile([128, 2, T], fp32r)
            vs_sb = big.tile([128, NTB, HPC, HD + 1], fp32r)

            xt_r = xt_d[:].rearrange("(k p) t -> p k t", p=128).bitcast(fp32r)
            xt_c = [None] * NCH
            # pcS[nj][h]: ctxT' drained to SBUF at end of chunk nj's attention
            pcS = [[None] * HPC for _ in range(NCH)]
            rsAll = [None] * NCH

            def load_xt(nj):
                xt_c[nj] = xtp.tile([128, NKB, 512], fp32r, tag="xt",
                                    name=f"xt{nj}")
                half = NKB // 2
                nc.sync.dma_start(out=xt_c[nj][:, 0:half, :],
                                  in_=xt_r[:, 0:half, nj * 512:(nj + 1) * 512])
                nc.sync.dma_start(out=xt_c[nj][:, half:, :],
                                  in_=xt_r[:, half:, nj * 512:(nj + 1) * 512])

            def qkv_halves(nj):
                """16 closures, each half a psum accumulation group (4 MMs)."""
                cs = slice(nj * 512, (nj + 1) * 512)
                quanta = []

                def make_qk(wsb, dst, mb):
                    pq = [None]

                    def go_a():
                        pq[0] = psum.tile([128, 512], fp32, tag="mm", bufs=4,
                                          name=f"pq{nj}{mb}")
                        for kb in range(4):
                            nc.tensor.matmul(
                                pq[0],
                                wsb[:, kb, mb * 128:(mb + 1) * 128],
                                xt_c[nj][:, kb, :],
                                start=(kb == 0), stop=False,
                            )

                    def go_b():
                        for kb in range(4, NKB):
                            nc.tensor.matmul(
                                pq[0],
                                wsb[:, kb, mb * 128:(mb + 1) * 128],
                                xt_c[nj][:, kb, :],
                                start=False, stop=(kb == NKB - 1),
                            )
                        nc.vector.tensor_copy(dst[:, mb, cs], pq[0])
                    return go_a, go_b

                def make_v(tb):
                    pv = [None]

                    def go_a():
                        pv[0] = psum.tile([128, 512], fp32, tag="mm", bufs=4,
                                          name=f"pv{tb}")
                        for kb in range(4):
                            nc.tensor.matmul(
                                pv[0][:, 0:DHC],
                                xt_c[nj][:, kb, (tb - 4 * nj) * 128:(tb - 4 * nj + 1) * 128],
                                wv_sb[:, kb, :],
                                start=(kb == 0), stop=False,
                            )

                    def go_b():
                        for kb in range(4, NKB):
                            nc.tensor.matmul(
                                pv[0][:, 0:DHC],
                                xt_c[nj][:, kb, (tb - 4 * nj) * 128:(tb - 4 * nj + 1) * 128],
                                wv_sb[:, kb, :],
                                start=False, stop=(kb == NKB - 1),
                            )
                        nc.vector.tensor_copy(
                            vs_sb[:, tb, :, 0:HD],
                            pv[0][:, 0:DHC].rearrange("p (h d) -> p h d", h=HPC),
                        )
                    return go_a, go_b

                for mb in range(2):
                    quanta.extend(make_qk(wq_sb, qt_sb, mb))
                for mb in range(2):
                    quanta.extend(make_qk(wk_sb, kt_sb, mb))
                for tb in range(4 * nj, 4 * nj + 4):
                    quanta.extend(make_v(tb))
                return quanta

            rcAll = [None] * NCH

            def prenorm_fill(nj):
                """two batched reciprocals cover all 4 heads' rowsums
                (heads live at 32-aligned partitions 0/32 of two tiles)."""
                def go():
                    rcAll[nj] = []
                    for half in range(2):
                        tmp32 = small.tile([33, 512], fp32, tag="tmp32",
                                           bufs=2, name=f"tmp{nj}{half}")
                        nc.vector.reciprocal(out=tmp32, in_=rsAll[nj][half])
                        for sub in range(2):
                            rch = small.tile([1, 512], fp32r, tag="rc", bufs=8,
                                             name=f"rc{nj}{half}{sub}")
                            with nc.allow_low_precision(reason="fp32r recip for PE bcast"):
                                nc.vector.tensor_copy(
                                    rch, tmp32[32 * sub:32 * sub + 1, :])
                            rcAll[nj].append(rch)
                return go

            def norm_fill(nj, h):
                """normalize head h of chunk nj from the SBUF-drained ctxT'."""
                def go():
                    mbh, ro = h >> 1, (h & 1) * 64
                    src = pcS[nj][h]
                    pb = psum.tile([64, 512], fp32, tag="mm", bufs=4,
                                   name=f"pb{nj}{h}")
                    nc.tensor.matmul(pb, cm_sb[0:1, 512:576], rcAll[nj][h],
                                     start=True, stop=True)
                    nc.vector.tensor_mul(
                        ct_sb[ro:ro + 64, mbh, nj * 512:(nj + 1) * 512],
                        src[0:64, :], pb)
                return go

            def outproj_fill(nj, tb):
                def go():
                    ot = outp.tile([128, D], fp32, tag="ot", name=f"ot{tb}")
                    for nk in range(2):
                        po = psum.tile([128, 512], fp32, tag="mm", bufs=4,
                                       name=f"po{tb}{nk}")
                        for mb in range(2):
                            nc.tensor.matmul(
                                po,
                                ct_sb[:, mb, tb * 128:(tb + 1) * 128],
                                wo_sb[:, mb, nk * 512:(nk + 1) * 512],
                                start=(mb == 0), stop=(mb == 1),
                            )
                        nc.vector.tensor_copy(ot[:, nk * 512:(nk + 1) * 512], po)
                    nc.sync.dma_start(out=out_d[tb * 128:(tb + 1) * 128, :],
                                      in_=ot)
                return go

            def norm_fills(nj):
                return [prenorm_fill(nj)] + [norm_fill(nj, h)
                                             for h in range(HPC)]

            def outproj_fills(nj):
                return [outproj_fill(nj, tb)
                        for tb in range(4 * nj, 4 * nj + 4)]

            # prologue: DMAs ordered so QKV(0) can start as early as
            # possible (wq then xt0 land first), then first chunk's QKV
            load_xt(0)
            nc.sync.dma_start(out=wk_sb, in_=wk_d[:].rearrange("(k p) n -> p k n", p=128).bitcast(fp32r))
            nc.sync.dma_start(out=wv_sb, in_=wv_d[:].rearrange("(k p) n -> p k n", p=128).bitcast(fp32r))
            load_xt(1)
            nc.sync.dma_start(out=cm_sb, in_=cm_d[:].bitcast(fp32r))
            nc.sync.dma_start(out=wo_sb, in_=wo_d[:].rearrange("(k p) n -> p k n", p=128).bitcast(fp32r))
            # ones column of v' (cmask cols 512.. are all 1.0, dtype fp32r)
            nc.vector.tensor_copy(
                vs_sb[:, :, :, 64],
                cm_sb[:, 512:512 + NTB * HPC].rearrange("p (a b) -> p a b", a=NTB),
            )
            for q in qkv_halves(0):
                q()

            for nj in range(NCH):
                nb = 4 * nj + 4     # causal: tk-blocks 0 .. nb-1
                if nj + 2 < NCH:
                    load_xt(nj + 2)
                # deferred fill work for this chunk's attention span:
                #   - next chunk's QKV projections
                #   - previous chunk's normalization
                #   - output projections delayed by TWO chunks (they have no
                #     downstream consumers, so park them where attention has
                #     the least other fill -- the late, widest chunks)
                fill = list(qkv_halves(nj + 1)) if nj + 1 < NCH else []
                if nj >= 1:
                    # norm after ALL qkv fills: its 3.3us DVE reciprocal must
                    # not sit in the DVE queue ahead of load-bearing copies
                    fill = fill + norm_fills(nj - 1)
                if nj == NCH - 1:
                    fill += (outproj_fills(0) + outproj_fills(1)
                             + outproj_fills(2))
                fi = 0

                total_iters = 2 * nb
                it = 0
                rsAll[nj] = []
                for hp in range(2):
                    heads = (2 * hp, 2 * hp + 1)
                    pcs2 = [psum.tile([65, 512], fp32, tag="acc", bufs=2,
                                      name=f"pc{nj}{h}") for h in heads]
                    prev = None
                    for i in range(nb):
                        m = i - 4 * nj
                        # causal window: diagonal blocks only need cols >= wm
                        # (m==3 keeps N>=256 to stay at full fp32r rate)
                        wm = 0 if m < 0 else (128 * m if m < 3 else 256)
                        cur = []
                        for k, h in enumerate(heads):
                            mbh, ro = h >> 1, (h & 1) * 64
                            ps = psum.tile([128, 512], fp32, tag="mm", bufs=4,
                                           name=f"ps{nj}{h}{i}")
                            nc.tensor.matmul(
                                ps[:, wm:512],
                                kt_sb[ro:ro + 64, mbh, i * 128:(i + 1) * 128],
                                qt_sb[ro:ro + 64, mbh, nj * 512 + wm:(nj + 1) * 512],
                                start=True, stop=True,
                            )
                            es = es_pool.tile([128, 512], fp32r, tag="es",
                                              name=f"es{nj}{h}{i}")
                            nc.scalar.activation(out=es[:, wm:512],
                                                 in_=ps[:, wm:512],
                                                 func=Exp, scale=0.125)
                            if m >= 0:
                                # only the diagonal 128 cols (plus, for m==3,
                                # the below-window cols) need masking
                                a = wm if m == 3 else 128 * m
                                nc.vector.tensor_mul(
                                    es[:, a:128 * m + 128],
                                    es[:, a:128 * m + 128],
                                    cm_sb[:, (3 - m) * 128 + a:512],
                                )
                            cur.append(es)
                        if prev is not None:
                            # ctx for iteration i-1: its exps had a full
                            # iteration of PE work to complete on ACT
                            pes, pwm, pi = prev
                            for k, h in enumerate(heads):
                                nc.tensor.matmul(
                                    pcs2[k][:, pwm:512],
                                    vs_sb[:, pi, h, :],
                                    pes[k][:, pwm:512],
                                    start=(pi == 0), stop=False,
                                )
                        prev = (cur, wm, i)
                        it += 1
                        while fi < min(len(fill),
                                       int(len(fill) * it / total_iters + 0.999)):
                            fill[fi]()
                            fi += 1
                    pes, pwm, pi = prev
                    for k, h in enumerate(heads):
                        nc.tensor.matmul(
                            pcs2[k][:, pwm:512],
                            vs_sb[:, pi, h, :],
                            pes[k][:, pwm:512],
                            start=(pi == 0), stop=True,
                        )
                    # drain this pass's ctxT' + rowsums to SBUF (ACT does the
                    # big copies; rowsums go to 32-aligned rows for the
                    # batched reciprocal)
                    rsh = small.tile([33, 512], fp32, tag="rsall", bufs=4,
                                     name=f"rsAll{nj}{hp}")
                    nc.gpsimd.memset(rsh, 1.0)
                    rsAll[nj].append(rsh)
                    for k, h in enumerate(heads):
                        dst = small.tile([64, 512], fp32, tag="pcs", bufs=8,
                                         name=f"pcS{nj}{h}")
                        nc.scalar.copy(dst, pcs2[k][0:64, :])
                        pcS[nj][h] = dst
                        nc.vector.tensor_copy(rsh[32 * k:32 * k + 1, :],
                                              pcs2[k][64:65, :])
                while fi < len(fill):
                    fill[fi]()
                    fi += 1

            # last chunk's tail has no next attention to hide in
            for go in norm_fills(NCH - 1) + outproj_fills(NCH - 1):
                go()

    nc.compile()
    return nc


def _causal_mask_block():
    # [128, 1024]: cols 0..383 = 0, cols 384..511 = upper-tri (p <= c-384),
    # cols 512.. = 1.  Slice [(3-m)*128 : (3-m)*128+512] masks a diagonal
    # tk-block at position m within a 512-wide tq chunk.
    m = np.zeros((128, 1024), np.float32)
    m[:, 512:] = 1.0
    m[:, 384:512] = np.triu(np.ones((128, 128), np.float32))
    return m


def _prepare_in_maps(x_q, Wq, Wk, Wv, Wo):
    x_q = np.asarray(x_q, np.float32)
    Wq = np.asarray(Wq, np.float32)
    Wk = np.asarray(Wk, np.float32)
    Wv = np.asarray(Wv, np.float32)
    Wo = np.asarray(Wo, np.float32)

    cmask = _causal_mask_block()
    xts = [np.ascontiguousarray(x_q[b].T) for b in range(B)]
    in_maps = []
    for c in range(NCORES):
        b, g = divmod(c, GROUPS)
        sl = slice(g * DHC, (g + 1) * DHC)
        in_maps.append({
            "xt": xts[b],
            "wq": np.ascontiguousarray(Wq[:, sl]),
            "wk": np.ascontiguousarray(Wk[:, sl]),
            "wv": np.ascontiguousarray(Wv[:, sl]),
            "wo": np.ascontiguousarray(Wo[sl, :]),
            "cmask": cmask,
        })
    return in_maps


def _gather(results):
    out = np.zeros((B, T, D), np.float32)
    for c in range(NCORES):
        out[c // GROUPS] += results[c]["out"]
    return out


def get_nc():
    if "nc" not in _CACHE:
        _CACHE["nc"] = _build()
    return _CACHE["nc"]


def kernel(x_q, Wq, Wk, Wv, Wo):
    from concourse.bass_utils import run_bass_kernel_spmd

    nc = get_nc()
    in_maps = _prepare_in_maps(x_q, Wq, Wk, Wv, Wo)
    res = run_bass_kernel_spmd(nc, in_maps, list(range(NCORES)))
    return _gather(res.results)
